# revision 14
# baseline (speedup 1.0000x reference)
"""Per-batch (block-diagonal) cross-attention kernel for Trainium2.

Each query row attends only to key/value rows with the same batch id
(ids in [0, 8), both coor arrays sorted). Batch b -> core b: every core
runs one dense attention block of ~1k queries x ~1k keys, C=64, no
collectives.

v3 design (per core; P=128, Qv = valid queries, Kp = padded keys,
nk = Kp/128, M = main cols, TW = Qv - M tail cols):

  The S matmul has contraction C=64 -- half the PE array. v3 runs it
  in 64x128 row-tiled mode: tile T0 (SBUF partitions 0-63) computes
  S^T for even k-tiles, tile T8 (partitions 64-127) for odd k-tiles,
  concurrently. Q^T is staged twice in SBUF (rows 0-63 and 64-127) so
  each row-tile streams its own copy; even k-tiles' K^T lives at
  partitions 0-63, odd k-tiles' at 64-127 (plus a low copy of every
  tile for the tail matmuls, which all run on T0 so their shared PSUM
  bank is never hit by two tiles at once).

  Per pass j (k-tiles A=2j, B=2j+1):
    - S^T(A) -> ps_A [128, M] (2 banks), S^T(B) -> ps_B (2 banks),
      both in 512-col chunks, T0/T8 interleaved so the array runs 2x.
    - tails: T0-only matmuls into ps_t [128, 2*TW] (1 bank).
    - exp: ACT exact exp for A chunks, DVE Schraudolph for B chunks
      (i16 = S*A + B bitcast to bf16, ~1.5% per-element, cancels in
      softmax normalization); one DVE op for both tails.
    - PV: 128-mode matmuls, kv tile stationary (65 cols: V + ones
      column for the softmax denominator), streaming P^T chunks + the
      tail chunk, accumulated over k-tiles in out^T PSUM chunks
      (c0/c1: [65,512] 1 bank each, c2: [65,TW] 1 bank).
  PSUM: 2+2+1+3 = 8 banks, S single-buffered (exp of pass j frees its
  banks before pass j+1's S needs them).

  Inputs go out on five queues in first-needed order (sync/tensor:
  K^T low/high, scalar/vector: Q^T low/high, gpsimd SWDGE: kv) so S
  of pass 0 starts as soon as the head chunks land and later passes
  never wait. A short warmup matmul train keeps the PE out of its low
  p-state until data arrives without delaying the first real matmul.

  Output: out^T [65, Qv] copied PSUM->SBUF as bf16 (c0 on ACT early,
  c1+c2 on DVE) and DMA'd on two rings; the host does the
  numerator/denominator divide in fp32 and the transpose back.
"""

import os
from contextlib import ExitStack

import numpy as np

import concourse.bacc as bacc
import concourse.mybir as mybir
import concourse.tile as tile
from concourse.bass_utils import run_bass_kernel_spmd

N_CORES = 8
C = 64
P = 128
KW = C + 1  # kv tile width (65: values + ones column)
SCALE = 1.0 / 8.0  # 1/sqrt(C)
F32 = mybir.dt.float32
BF16 = mybir.dt.bfloat16
I16 = mybir.dt.int16

# Schraudolph exp approximation in bf16: exp(s/8) ~= bitcast_bf16(
# int16(s * A + B)). A folds the 1/sqrt(C) score scale into 2^7/ln2.
SCH_A = 184.66496736312366 / 8.0
SCH_B = 16256.0 - 7.42

BANK_F32 = 512  # fp32 elements per PSUM bank (2KB)

_LAST_RUN = {}


def _round_up(x: int, m: int) -> int:
    return -(-x // m) * m


def _emit3(ctx: ExitStack, tc: "tile.TileContext", out_ap, qt2_ap, ktd_ap,
           kv_ap, Qp: int, Kp: int, Qv: int):
    nc = tc.nc
    nk = Kp // P
    n_hi = nk // 2  # number of odd k-tiles (stationaries for T8)
    npass = (nk + 1) // 2

    M = min(Qv, 2 * BANK_F32)
    TW = Qv - M
    m_chunks = [(0, min(M, BANK_F32))]
    if M > BANK_F32:
        m_chunks.append((BANK_F32, M - BANK_F32))
    ncn = len(m_chunks)
    c0w = m_chunks[0][1]

    big = ctx.enter_context(tc.tile_pool(name="big", bufs=1))
    psum_s = ctx.enter_context(tc.tile_pool(name="pss", bufs=1, space="PSUM"))
    psum_o = ctx.enter_context(tc.tile_pool(name="pso", bufs=1, space="PSUM"))
    if TW > 0:
        psum_t = ctx.enter_context(
            tc.tile_pool(name="pst", bufs=1, space="PSUM"))

    # One SBUF tile per DMA op: a tile with several DMA writers makes
    # every reader wait for the LAST writer, gating the whole pipeline
    # on the biggest transfer.
    km = max(P, n_hi * P)  # full-height region end (hi rows packed)
    kt_head = big.tile([P, P], BF16, tag="kth", name="kt_head")
    kt_restA = (big.tile([P, km - P], BF16, tag="ktra", name="kt_restA")
                if km > P else None)
    kt_restB = (big.tile([C, Kp - km], BF16, tag="ktrb", name="kt_restB")
                if Kp > km else None)
    qt_c0 = big.tile([P, c0w], BF16, tag="qt0", name="qt_c0")
    qt_rest = big.tile([P, Qp - c0w], BF16, tag="qtr", name="qt_rest")
    kv_all = big.tile([P, nk * KW], BF16, tag="kv_all", name="kv_all")
    warm = big.tile([C, 512], BF16, tag="warm", name="warm")

    # warm memset rides the idle DVE queue so the gpsimd queue starts
    # the kv DMA immediately.
    nc.vector.memset(warm[:], 0.0)

    # Input DMAs on the three DMA-capable queues, first-needed-first.
    # Beyond the packed hi region only the low rows are transferred.
    nc.sync.dma_start(kt_head[:], ktd_ap[:, 0:P])
    nc.scalar.dma_start(qt_c0[:], qt2_ap[:, 0:c0w])
    if kt_restA is not None:
        nc.sync.dma_start(kt_restA[:], ktd_ap[:, P:km])
    if c0w < Qp:
        nc.scalar.dma_start(qt_rest[:], qt2_ap[:, c0w:Qp])
    if kt_restB is not None:
        nc.sync.dma_start(kt_restB[:], ktd_ap[0:C, km:Kp])
    nc.gpsimd.dma_start(kv_all[:], kv_ap[:, :])

    def kt_lo(t):  # low-rows K^T of k-tile t (T0 mains + all tails)
        if t == 0:
            return kt_head[0:C, 0:P]
        if t * P < km:
            return kt_restA[0:C, (t - 1) * P:t * P]
        return kt_restB[:, t * P - km:(t + 1) * P - km]

    def kta_s(j):  # T0 stationary: even k-tile 2j
        return kt_lo(2 * j)

    def ktb_hi_s(j):  # T8 stationary: odd k-tile 2j+1, high rows
        return (kt_head[C:P, 0:P] if j == 0
                else kt_restA[C:P, (j - 1) * P:j * P])

    def ktb_lo_s(j):  # low copy of odd k-tile 2j+1 (T0 tail matmul)
        return kt_lo(2 * j + 1)

    def qt_sl(rows, off, w):  # query cols [off:off+w], T0 or T8 rows
        r = slice(0, C) if rows == 0 else slice(C, P)
        if off + w <= c0w:
            return qt_c0[r, off:off + w]
        return qt_rest[r, off - c0w:off - c0w + w]

    # Per-chunk P^T tiles: tile-granular dependency tracking means a
    # single [P, M] tile would stall each consumer on every producer.
    pt_c = [[big.tile([P, w], BF16, tag=f"pt{k}c{i}", name=f"pt{k}c{i}")
             for i, (off, w) in enumerate(m_chunks)] for k in range(nk)]
    if TW > 0:
        pt_tails = big.tile([P, nk * TW], BF16, tag="ptt", name="ptt")

    pso_c = [psum_o.tile([KW, w], F32, tag=f"pso{i}", name=f"pso{i}")
             for i, (off, w) in enumerate(m_chunks)]
    if TW > 0:
        pso_tl = psum_o.tile([KW, TW], F32, tag="psotl", name="psotl")
    pso_started = {}

    # PE continuity: the 2.4GHz p-state needs ~3us of gap-free PE
    # execution and resets on ~300ns idle. Warmup matmuls bridge the
    # input-DMA head (into pso_c[0], safe: the first real PV write
    # restarts the accumulation group); standalone LDWEIGHTS groups
    # bridge the exp-latency bubble between phases without needing a
    # PSUM target (each real matmul reloads its own weights anyway).
    def emit_fill(n, src_tile=None):
        for _ in range(n):
            nc.tensor.ldweights(
                warm[:, 0:P] if src_tile is None else src_tile,
                tile_position=(0, 0),
            )

    n_warm = int(os.environ.get("XATTN_WARMUP", "20"))
    for _ in range(n_warm):
        nc.tensor.matmul(
            pso_c[0][0:C, 0:128], lhsT=warm[:, 0:C], rhs=warm[:, 0:128],
            start=True, stop=True, skip_group_check=True,
            tile_position=(0, 0),
        )

    def emit_s_mains(j):
        """S^T mains of pass j: per-chunk PSUM tiles, T0 (even k-tile)
        and T8 (odd) interleaved per chunk so both row-tiles stream
        concurrently."""
        B = 2 * j + 1 if 2 * j + 1 < nk else None
        ps_A = [psum_s.tile([P, w], F32, tag=f"psa{i}", name=f"psA{j}c{i}")
                for i, (off, w) in enumerate(m_chunks)]
        ps_B = ([psum_s.tile([P, w], F32, tag=f"psb{i}", name=f"psB{j}c{i}")
                 for i, (off, w) in enumerate(m_chunks)]
                if B is not None else None)
        for i, (off, w) in enumerate(m_chunks):
            nc.tensor.matmul(
                ps_A[i][:], lhsT=kta_s(j), rhs=qt_sl(0, off, w),
                start=True, stop=True, tile_position=(0, 0),
            )
            if B is not None:
                nc.tensor.matmul(
                    ps_B[i][:], lhsT=ktb_hi_s(j), rhs=qt_sl(1, off, w),
                    start=True, stop=True, tile_position=(C, 0),
                )
        return ps_A, ps_B

    def emit_s_tails(j):
        """S^T tails of pass j, all on T0 (the odd k-tile via its low
        copy) so the shared tails bank never sees two row-tiles at
        once."""
        if TW <= 0:
            return None
        B = 2 * j + 1 if 2 * j + 1 < nk else None
        nt = 2 if B is not None else 1
        ps_t = psum_t.tile([P, nt * TW], F32, tag="pst", name=f"psT{j}")
        nc.tensor.matmul(
            ps_t[:, 0:TW], lhsT=kta_s(j), rhs=qt_sl(0, M, TW),
            start=True, stop=True, tile_position=(0, 0),
        )
        if B is not None:
            nc.tensor.matmul(
                ps_t[:, TW:2 * TW], lhsT=ktb_lo_s(j), rhs=qt_sl(0, M, TW),
                start=True, stop=True, tile_position=(0, 0),
            )
        return ps_t

    def emit_exp(j, ps_A, ps_B, ps_t):
        """exp: ACT exact for A chunks (+ the pass's tails), DVE
        Schraudolph for B chunks; an A-only last pass splits A across
        both engines at the chunk boundary."""
        A = 2 * j
        B = 2 * j + 1 if 2 * j + 1 < nk else None
        for i in range(ncn):
            if i == 1 and B is None:
                nc.vector.tensor_scalar(
                    pt_c[A][i][:].bitcast(I16), ps_A[i][:],
                    SCH_A, SCH_B, mybir.AluOpType.mult, mybir.AluOpType.add,
                )
                continue
            nc.scalar.activation(
                pt_c[A][i][:], ps_A[i][:],
                mybir.ActivationFunctionType.Exp, scale=SCALE,
            )
            if B is not None:
                nc.vector.tensor_scalar(
                    pt_c[B][i][:].bitcast(I16), ps_B[i][:],
                    SCH_A, SCH_B, mybir.AluOpType.mult, mybir.AluOpType.add,
                )
        if TW > 0:
            nt = 2 if B is not None else 1
            nc.scalar.activation(
                pt_tails[:, A * TW:(A + nt) * TW], ps_t[:],
                mybir.ActivationFunctionType.Exp, scale=SCALE,
            )

    def pv_mm(kti, out_tile, key, rhs, stop):
        first = key not in pso_started
        pso_started[key] = True
        nc.tensor.matmul(
            out_tile[:], lhsT=kv_all[:, kti * KW:(kti + 1) * KW], rhs=rhs,
            start=first, stop=stop, skip_group_check=True,
        )

    def emit_pv(j):
        """PV of pair j (128-mode), k-tile-grouped so each kv stationary
        loads once. All exp inputs completed a pass ago. The final
        k-tile is skipped here and fused with the output stage."""
        A = 2 * j
        B = 2 * j + 1 if 2 * j + 1 < nk else None
        for k in (A, B):
            if k is None or k == nk - 1:
                continue
            for i in range(ncn):
                pv_mm(k, pso_c[i], i, pt_c[k][i][:], False)
            if TW > 0:
                pv_mm(k, pso_tl, "tl", pt_tails[:, k * TW:(k + 1) * TW],
                      False)

    # Skewed pipeline with the PE queue order pinned by tile_wait_until
    # (a scheduler-sim-only earliest-start hint, no hardware delay):
    # S(0) | fill | S(1)+tails(0) | PV(0)+fill | S(2)+tails(1) | ...
    # exp(j) runs on ACT/DVE during PV(j-1) so PV never waits; fill
    # groups bridge the exp round-trip so the PE never idles.
    # Waits pin ONLY the PV groups (whose inputs completed a pass ago,
    # so the scheduler-sim's threshold batching is harmless there). S
    # groups stay unwaited: a waited instruction's semaphore thresholds
    # are derived from its simulated-time position, which folds every
    # earlier-simulated DMA/exp completion into its wait -- gating S on
    # unrelated transfers.
    n_fill = int(os.environ.get("XATTN_FILL_N", "4"))
    n_fill0 = int(os.environ.get("XATTN_FILL0_N", "6"))
    w0 = float(os.environ.get("XATTN_W0", "4.0"))     # us, first PV group
    wstep = float(os.environ.get("XATTN_WSTEP", "2.4"))  # us per pass

    psA0, psB0 = emit_s_mains(0)
    # fill0 reads kt_head so it becomes ready with the first DMA and
    # bridges the input-DMA head alongside S(0).
    emit_fill(n_fill0, kt_head[0:C, 0:P])
    ps_prev = None
    for j in range(1, npass):
        ps_t_prev = emit_s_tails(j - 1)
        if j == 1:
            emit_exp(0, psA0, psB0, ps_t_prev)
        else:
            emit_exp(j - 1, ps_prev[0], ps_prev[1], ps_t_prev)
        psj = emit_s_mains(j)
        with tc.tile_wait_until((w0 + wstep * (j - 1)) * 1e-3):
            emit_pv(j - 1)
            emit_fill(n_fill)
        ps_prev = psj
    ms = (w0 + wstep * (npass - 1)) * 1e-3
    ps_t_last = emit_s_tails(npass - 1)
    if npass == 1:
        emit_exp(0, psA0, psB0, ps_t_last)
    else:
        emit_exp(npass - 1, ps_prev[0], ps_prev[1], ps_t_last)
    with tc.tile_wait_until(ms):
        emit_pv(npass - 1)

    # Final k-tile PV fused with the output stage: c0 closes first so
    # its cast + DMA overlap the c1/tail matmuls. DMA cannot read PSUM,
    # so stage through SBUF as bf16; host divides num/den in fp32.
    lk = nk - 1
    ow0 = m_chunks[0][1]
    obuf0 = big.tile([KW, ow0], BF16, tag="obuf0", name="obuf0")
    with tc.tile_wait_until(ms):
        pv_mm(lk, pso_c[0], 0, pt_c[lk][0][:], True)
    nc.scalar.activation(
        obuf0[:], pso_c[0][:], mybir.ActivationFunctionType.Copy,
    )
    nc.sync.dma_start(out_ap[:, 0:ow0], obuf0[:])
    if Qv > ow0:
        obuf1 = big.tile([KW, Qv - ow0], BF16, tag="obuf1", name="obuf1")
        if ncn > 1:
            w1 = m_chunks[1][1]
            with tc.tile_wait_until(ms):
                pv_mm(lk, pso_c[1], 1, pt_c[lk][1][:], True)
            nc.vector.tensor_copy(obuf1[:, 0:w1], pso_c[1][:])
        if TW > 0:
            with tc.tile_wait_until(ms):
                pv_mm(lk, pso_tl, "tl", pt_tails[:, lk * TW:(lk + 1) * TW],
                      True)
            nc.vector.tensor_copy(obuf1[:, M - ow0:Qv - ow0], pso_tl[:])
        nc.gpsimd.dma_start(out_ap[:, ow0:Qv], obuf1[:])


def build_program(Qp: int, Kp: int, Qv: int):
    nc = bacc.Bacc(
        trn_type="TRN2",
        target_bir_lowering=False,
        debug=False,
        num_devices=N_CORES,
    )
    nk = Kp // P
    qt2_ap = nc.dram_tensor("qt2", [P, Qp], BF16, kind="ExternalInput").ap()
    ktd_ap = nc.dram_tensor("ktd", [P, Kp], BF16, kind="ExternalInput").ap()
    kv_ap = nc.dram_tensor("kv", [P, nk * KW], BF16, kind="ExternalInput").ap()
    out_ap = nc.dram_tensor("outT", [KW, Qv], BF16, kind="ExternalOutput").ap()
    with tile.TileContext(nc) as tc, ExitStack() as ctx:
        _emit3(ctx, tc, out_ap, qt2_ap, ktd_ap, kv_ap, Qp, Kp, Qv)
    nc.compile()
    return nc


def shard_inputs(query, key_value, query_coors, key_value_coors):
    import ml_dtypes
    query = np.ascontiguousarray(np.asarray(query), dtype=np.float32)
    key_value = np.ascontiguousarray(np.asarray(key_value), dtype=np.float32)
    qc = np.asarray(query_coors).astype(np.int64)
    kc = np.asarray(key_value_coors).astype(np.int64)
    B = N_CORES
    ids = np.arange(B)
    qs = np.searchsorted(qc, ids, side="left")
    qe = np.searchsorted(qc, ids, side="right")
    ks = np.searchsorted(kc, ids, side="left")
    ke = np.searchsorted(kc, ids, side="right")
    qcnt, kcnt = qe - qs, ke - ks
    Qp = max(_round_up(int(qcnt.max()), P), P)
    Kp = max(_round_up(int(kcnt.max()), P), P)
    Qv = min(_round_up(int(qcnt.max()), 4), Qp)
    nk = Kp // P
    n_hi = nk // 2
    in_maps = []
    for b in range(B):
        qsh = np.zeros((Qp, C), np.float32)
        qsh[: qcnt[b]] = query[qs[b]: qe[b]]
        kvsh = np.zeros((Kp, KW), np.float32)
        kvsh[: kcnt[b], :C] = key_value[ks[b]: ke[b]]
        kvsh[: kcnt[b], C] = 1.0
        kt = kvsh[:, :C].T  # [C, Kp]
        # ktd: rows 0-63 = K^T of every k-tile (T0 mains + tails), rows
        # 64-127 = odd k-tiles' K^T packed at cols [j*P] (T8 mains).
        ktd = np.zeros((P, Kp), np.float32)
        ktd[0:C, :] = kt
        for t in range(n_hi):
            ktd[C:P, t * P:(t + 1) * P] = kt[:, (2 * t + 1) * P:(2 * t + 2) * P]
        # qt2: Q^T duplicated to both row-tile halves.
        qt2 = np.concatenate([qsh.T, qsh.T], axis=0)
        kv_il = kvsh.reshape(nk, P, KW).transpose(1, 0, 2).reshape(P, nk * KW)
        in_maps.append({
            "qt2": np.ascontiguousarray(qt2.astype(ml_dtypes.bfloat16)),
            "ktd": np.ascontiguousarray(ktd.astype(ml_dtypes.bfloat16)),
            "kv": np.ascontiguousarray(kv_il.astype(ml_dtypes.bfloat16)),
        })
    return in_maps, (qs, qe, qcnt), Qp, Kp, Qv


def kernel(query, key_value, query_coors, key_value_coors):
    in_maps, (qs, qe, qcnt), Qp, Kp, Qv = shard_inputs(
        query, key_value, query_coors, key_value_coors
    )
    nc = build_program(Qp, Kp, Qv)
    trace = bool(os.environ.get("XATTN_TRACE"))
    res = run_bass_kernel_spmd(
        nc, in_maps, list(range(N_CORES)), trace=trace,
        trace_cores=list(range(N_CORES)) if trace else None,
    )
    _LAST_RUN["exec_time_ns"] = res.exec_time_ns
    _LAST_RUN["mean_exec_time_ns"] = res.mean_exec_time_ns
    _LAST_RUN["trace"] = res.instructions_and_trace
    _LAST_RUN["results"] = res
    N1 = np.asarray(query).shape[0]
    out = np.zeros((N1, C), np.float32)
    for b in range(N_CORES):
        ot = np.asarray(res.results[b]["outT"], dtype=np.float32)  # [65, Qv]
        n = int(qcnt[b])
        num = ot[:C, :n]
        den = ot[C, :n]
        out[qs[b]: qe[b]] = (num / den[None, :]).T
    return out


# revision 15
# speedup vs baseline: 1.0615x; 1.0615x over previous
"""Per-batch (block-diagonal) cross-attention kernel for Trainium2.

Each query row attends only to key/value rows with the same batch id
(ids in [0, 8), both coor arrays sorted). Batch b -> core b: every core
runs one dense attention block of ~1k queries x ~1k keys, C=64, no
collectives.

v3 design (per core; P=128, Qv = valid queries, Kp = padded keys,
nk = Kp/128, M = main cols, TW = Qv - M tail cols):

  The S matmul has contraction C=64 -- half the PE array. v3 runs it
  in 64x128 row-tiled mode: tile T0 (SBUF partitions 0-63) computes
  S^T for even k-tiles, tile T8 (partitions 64-127) for odd k-tiles,
  concurrently. Q^T is staged twice in SBUF (rows 0-63 and 64-127) so
  each row-tile streams its own copy; even k-tiles' K^T lives at
  partitions 0-63, odd k-tiles' at 64-127 (plus a low copy of every
  tile for the tail matmuls, which all run on T0 so their shared PSUM
  bank is never hit by two tiles at once).

  Per pass j (k-tiles A=2j, B=2j+1):
    - S^T(A) -> ps_A [128, M] (2 banks), S^T(B) -> ps_B (2 banks),
      both in 512-col chunks, T0/T8 interleaved so the array runs 2x.
    - tails: T0-only matmuls into ps_t [128, 2*TW] (1 bank).
    - exp: ACT exact exp for A chunks, DVE Schraudolph for B chunks
      (i16 = S*A + B bitcast to bf16, ~1.5% per-element, cancels in
      softmax normalization); one DVE op for both tails.
    - PV: 128-mode matmuls, kv tile stationary (65 cols: V + ones
      column for the softmax denominator), streaming P^T chunks + the
      tail chunk, accumulated over k-tiles in out^T PSUM chunks
      (c0/c1: [65,512] 1 bank each, c2: [65,TW] 1 bank).
  PSUM: 2+2+1+3 = 8 banks, S single-buffered (exp of pass j frees its
  banks before pass j+1's S needs them).

  Inputs go out on five queues in first-needed order (sync/tensor:
  K^T low/high, scalar/vector: Q^T low/high, gpsimd SWDGE: kv) so S
  of pass 0 starts as soon as the head chunks land and later passes
  never wait. A short warmup matmul train keeps the PE out of its low
  p-state until data arrives without delaying the first real matmul.

  Output: out^T [65, Qv] copied PSUM->SBUF as bf16 (c0 on ACT early,
  c1+c2 on DVE) and DMA'd on two rings; the host does the
  numerator/denominator divide in fp32 and the transpose back.
"""

import os
from contextlib import ExitStack

import numpy as np

import concourse.bacc as bacc
import concourse.mybir as mybir
import concourse.tile as tile
from concourse.bass_utils import run_bass_kernel_spmd

N_CORES = 8
C = 64
P = 128
KW = C + 1  # kv tile width (65: values + ones column)
SCALE = 1.0 / 8.0  # 1/sqrt(C)
F32 = mybir.dt.float32
BF16 = mybir.dt.bfloat16
I16 = mybir.dt.int16

# Schraudolph exp approximation in bf16: exp(s/8) ~= bitcast_bf16(
# int16(s * A + B)). A folds the 1/sqrt(C) score scale into 2^7/ln2.
SCH_A = 184.66496736312366 / 8.0
SCH_B = 16256.0 - 7.42

BANK_F32 = 512  # fp32 elements per PSUM bank (2KB)

_LAST_RUN = {}


def _round_up(x: int, m: int) -> int:
    return -(-x // m) * m


def _emit3(ctx: ExitStack, tc: "tile.TileContext", out_ap, qt2_ap, ktd_ap,
           kv_ap, Qp: int, Kp: int, Qv: int):
    nc = tc.nc
    nk = Kp // P
    n_hi = nk // 2  # number of odd k-tiles (stationaries for T8)
    npass = (nk + 1) // 2

    M = min(Qv, 2 * BANK_F32)
    TW = Qv - M
    m_chunks = [(0, min(M, BANK_F32))]
    if M > BANK_F32:
        m_chunks.append((BANK_F32, M - BANK_F32))
    ncn = len(m_chunks)
    c0w = m_chunks[0][1]

    big = ctx.enter_context(tc.tile_pool(name="big", bufs=1))
    psum_s = ctx.enter_context(tc.tile_pool(name="pss", bufs=1, space="PSUM"))
    psum_o = ctx.enter_context(tc.tile_pool(name="pso", bufs=1, space="PSUM"))
    if TW > 0:
        psum_t = ctx.enter_context(
            tc.tile_pool(name="pst", bufs=1, space="PSUM"))

    # One SBUF tile per DMA op: a tile with several DMA writers makes
    # every reader wait for the LAST writer, gating the whole pipeline
    # on the biggest transfer.
    km = max(P, n_hi * P)  # full-height region end (hi rows packed)
    kt_head = big.tile([P, P], BF16, tag="kth", name="kt_head")
    kt_restA = (big.tile([P, km - P], BF16, tag="ktra", name="kt_restA")
                if km > P else None)
    kt_restB = (big.tile([C, Kp - km], BF16, tag="ktrb", name="kt_restB")
                if Kp > km else None)
    qt_c0 = big.tile([P, c0w], BF16, tag="qt0", name="qt_c0")
    qt_rest = big.tile([P, Qp - c0w], BF16, tag="qtr", name="qt_rest")
    kv_all = big.tile([P, nk * KW], BF16, tag="kv_all", name="kv_all")
    warm = big.tile([C, 512], BF16, tag="warm", name="warm")

    # warm memset rides the idle DVE queue so the gpsimd queue starts
    # the kv DMA immediately.
    nc.vector.memset(warm[:], 0.0)

    # Input DMAs on the three DMA-capable queues, first-needed-first.
    # Beyond the packed hi region only the low rows are transferred.
    nc.sync.dma_start(kt_head[:], ktd_ap[:, 0:P])
    nc.scalar.dma_start(qt_c0[:], qt2_ap[:, 0:c0w])
    if c0w < Qp:
        nc.sync.dma_start(qt_rest[:], qt2_ap[:, c0w:Qp])
    if kt_restA is not None:
        nc.sync.dma_start(kt_restA[:], ktd_ap[:, P:km])
    if kt_restB is not None:
        nc.scalar.dma_start(kt_restB[:], ktd_ap[0:C, km:Kp])
    nc.gpsimd.dma_start(kv_all[:], kv_ap[:, :])

    def kt_lo(t):  # low-rows K^T of k-tile t (T0 mains + all tails)
        if t == 0:
            return kt_head[0:C, 0:P]
        if t * P < km:
            return kt_restA[0:C, (t - 1) * P:t * P]
        return kt_restB[:, t * P - km:(t + 1) * P - km]

    def kta_s(j):  # T0 stationary: even k-tile 2j
        return kt_lo(2 * j)

    def ktb_hi_s(j):  # T8 stationary: odd k-tile 2j+1, high rows
        return (kt_head[C:P, 0:P] if j == 0
                else kt_restA[C:P, (j - 1) * P:j * P])

    def ktb_lo_s(j):  # low copy of odd k-tile 2j+1 (T0 tail matmul)
        return kt_lo(2 * j + 1)

    def qt_sl(rows, off, w):  # query cols [off:off+w], T0 or T8 rows
        r = slice(0, C) if rows == 0 else slice(C, P)
        if off + w <= c0w:
            return qt_c0[r, off:off + w]
        return qt_rest[r, off - c0w:off - c0w + w]

    # Per-chunk P^T tiles: tile-granular dependency tracking means a
    # single [P, M] tile would stall each consumer on every producer.
    pt_c = [[big.tile([P, w], BF16, tag=f"pt{k}c{i}", name=f"pt{k}c{i}")
             for i, (off, w) in enumerate(m_chunks)] for k in range(nk)]
    if TW > 0:
        pt_tails = big.tile([P, nk * TW], BF16, tag="ptt", name="ptt")

    pso_c = [psum_o.tile([KW, w], F32, tag=f"pso{i}", name=f"pso{i}")
             for i, (off, w) in enumerate(m_chunks)]
    if TW > 0:
        pso_tl = psum_o.tile([KW, TW], F32, tag="psotl", name="psotl")
    pso_started = {}

    # PE continuity: the 2.4GHz p-state needs ~3us of gap-free PE
    # execution and resets on ~300ns idle. Warmup matmuls bridge the
    # input-DMA head (into pso_c[0], safe: the first real PV write
    # restarts the accumulation group); standalone LDWEIGHTS groups
    # bridge the exp-latency bubble between phases without needing a
    # PSUM target (each real matmul reloads its own weights anyway).
    def emit_fill(n, src_tile=None):
        for _ in range(n):
            nc.tensor.ldweights(
                warm[:, 0:P] if src_tile is None else src_tile,
                tile_position=(0, 0),
            )

    n_warm = int(os.environ.get("XATTN_WARMUP", "20"))
    for _ in range(n_warm):
        nc.tensor.matmul(
            pso_c[0][0:C, 0:128], lhsT=warm[:, 0:C], rhs=warm[:, 0:128],
            start=True, stop=True, skip_group_check=True,
            tile_position=(0, 0),
        )

    def emit_s_mains(j):
        """S^T mains of pass j: per-chunk PSUM tiles, T0 (even k-tile)
        and T8 (odd) interleaved per chunk so both row-tiles stream
        concurrently."""
        B = 2 * j + 1 if 2 * j + 1 < nk else None
        ps_A = [psum_s.tile([P, w], F32, tag=f"psa{i}", name=f"psA{j}c{i}")
                for i, (off, w) in enumerate(m_chunks)]
        ps_B = ([psum_s.tile([P, w], F32, tag=f"psb{i}", name=f"psB{j}c{i}")
                 for i, (off, w) in enumerate(m_chunks)]
                if B is not None else None)
        for i, (off, w) in enumerate(m_chunks):
            nc.tensor.matmul(
                ps_A[i][:], lhsT=kta_s(j), rhs=qt_sl(0, off, w),
                start=True, stop=True, tile_position=(0, 0),
            )
            if B is not None:
                nc.tensor.matmul(
                    ps_B[i][:], lhsT=ktb_hi_s(j), rhs=qt_sl(1, off, w),
                    start=True, stop=True, tile_position=(C, 0),
                )
        return ps_A, ps_B

    def emit_s_tails(j):
        """S^T tails of pass j, all on T0 (the odd k-tile via its low
        copy) so the shared tails bank never sees two row-tiles at
        once."""
        if TW <= 0:
            return None
        B = 2 * j + 1 if 2 * j + 1 < nk else None
        nt = 2 if B is not None else 1
        ps_t = psum_t.tile([P, nt * TW], F32, tag="pst", name=f"psT{j}")
        nc.tensor.matmul(
            ps_t[:, 0:TW], lhsT=kta_s(j), rhs=qt_sl(0, M, TW),
            start=True, stop=True, tile_position=(0, 0),
        )
        if B is not None:
            nc.tensor.matmul(
                ps_t[:, TW:2 * TW], lhsT=ktb_lo_s(j), rhs=qt_sl(0, M, TW),
                start=True, stop=True, tile_position=(0, 0),
            )
        return ps_t

    def emit_exp(j, ps_A, ps_B, ps_t):
        """exp: ACT exact for A chunks (+ the pass's tails), DVE
        Schraudolph for B chunks; an A-only last pass splits A across
        both engines at the chunk boundary."""
        A = 2 * j
        B = 2 * j + 1 if 2 * j + 1 < nk else None
        if TW > 0:
            # tails first: frees the shared tails bank before the next
            # pass's tail matmuls want it.
            nt = 2 if B is not None else 1
            nc.scalar.activation(
                pt_tails[:, A * TW:(A + nt) * TW], ps_t[:],
                mybir.ActivationFunctionType.Exp, scale=SCALE,
            )
        for i in range(ncn):
            if i == 1 and B is None:
                nc.vector.tensor_scalar(
                    pt_c[A][i][:].bitcast(I16), ps_A[i][:],
                    SCH_A, SCH_B, mybir.AluOpType.mult, mybir.AluOpType.add,
                )
                continue
            nc.scalar.activation(
                pt_c[A][i][:], ps_A[i][:],
                mybir.ActivationFunctionType.Exp, scale=SCALE,
            )
            if B is not None:
                nc.vector.tensor_scalar(
                    pt_c[B][i][:].bitcast(I16), ps_B[i][:],
                    SCH_A, SCH_B, mybir.AluOpType.mult, mybir.AluOpType.add,
                )

    def pv_mm(kti, out_tile, key, rhs, stop):
        first = key not in pso_started
        pso_started[key] = True
        nc.tensor.matmul(
            out_tile[:], lhsT=kv_all[:, kti * KW:(kti + 1) * KW], rhs=rhs,
            start=first, stop=stop, skip_group_check=True,
        )

    def emit_pv(j):
        """PV of pair j (128-mode), k-tile-grouped so each kv stationary
        loads once. All exp inputs completed a pass ago. The final
        k-tile is skipped here and fused with the output stage."""
        A = 2 * j
        B = 2 * j + 1 if 2 * j + 1 < nk else None
        for k in (A, B):
            if k is None or k == nk - 1:
                continue
            for i in range(ncn):
                pv_mm(k, pso_c[i], i, pt_c[k][i][:], False)
            if TW > 0:
                pv_mm(k, pso_tl, "tl", pt_tails[:, k * TW:(k + 1) * TW],
                      False)

    # Skewed pipeline with the PE queue order pinned by tile_wait_until
    # (a scheduler-sim-only earliest-start hint, no hardware delay):
    # S(0) | fill | S(1)+tails(0) | PV(0)+fill | S(2)+tails(1) | ...
    # exp(j) runs on ACT/DVE during PV(j-1) so PV never waits; fill
    # groups bridge the exp round-trip so the PE never idles.
    # Waits pin ONLY the PV groups (whose inputs completed a pass ago,
    # so the scheduler-sim's threshold batching is harmless there). S
    # groups stay unwaited: a waited instruction's semaphore thresholds
    # are derived from its simulated-time position, which folds every
    # earlier-simulated DMA/exp completion into its wait -- gating S on
    # unrelated transfers.
    n_fill = int(os.environ.get("XATTN_FILL_N", "4"))
    n_fill0 = int(os.environ.get("XATTN_FILL0_N", "6"))
    w0 = float(os.environ.get("XATTN_W0", "6.0"))     # us, first PV group
    wstep = float(os.environ.get("XATTN_WSTEP", "2.4"))  # us per pass

    psA0, psB0 = emit_s_mains(0)
    # fill0 reads kt_head so it becomes ready with the first DMA and
    # bridges the input-DMA head alongside S(0).
    emit_fill(n_fill0, kt_head[0:C, 0:P])
    ps_prev = None
    for j in range(1, npass):
        ps_t_prev = emit_s_tails(j - 1)
        if j == 1:
            emit_exp(0, psA0, psB0, ps_t_prev)
        else:
            emit_exp(j - 1, ps_prev[0], ps_prev[1], ps_t_prev)
        psj = emit_s_mains(j)
        with tc.tile_wait_until((w0 + wstep * (j - 1)) * 1e-3):
            emit_pv(j - 1)
            emit_fill(n_fill)
        ps_prev = psj
    ms = (w0 + wstep * (npass - 1)) * 1e-3
    ps_t_last = emit_s_tails(npass - 1)
    if npass == 1:
        emit_exp(0, psA0, psB0, ps_t_last)
    else:
        emit_exp(npass - 1, ps_prev[0], ps_prev[1], ps_t_last)
    with tc.tile_wait_until(ms):
        emit_pv(npass - 1)

    # Final k-tile PV fused with the output stage: c0 closes first so
    # its cast + DMA overlap the c1/tail matmuls. DMA cannot read PSUM,
    # so stage through SBUF as bf16; host divides num/den in fp32.
    lk = nk - 1
    ow0 = m_chunks[0][1]
    obuf0 = big.tile([KW, ow0], BF16, tag="obuf0", name="obuf0")
    with tc.tile_wait_until(ms):
        pv_mm(lk, pso_c[0], 0, pt_c[lk][0][:], True)
    nc.scalar.activation(
        obuf0[:], pso_c[0][:], mybir.ActivationFunctionType.Copy,
    )
    nc.sync.dma_start(out_ap[:, 0:ow0], obuf0[:])
    if Qv > ow0:
        obuf1 = big.tile([KW, Qv - ow0], BF16, tag="obuf1", name="obuf1")
        if ncn > 1:
            w1 = m_chunks[1][1]
            with tc.tile_wait_until(ms):
                pv_mm(lk, pso_c[1], 1, pt_c[lk][1][:], True)
            nc.vector.tensor_copy(obuf1[:, 0:w1], pso_c[1][:])
        if TW > 0:
            with tc.tile_wait_until(ms):
                pv_mm(lk, pso_tl, "tl", pt_tails[:, lk * TW:(lk + 1) * TW],
                      True)
            nc.vector.tensor_copy(obuf1[:, M - ow0:Qv - ow0], pso_tl[:])
        nc.gpsimd.dma_start(out_ap[:, ow0:Qv], obuf1[:])


def build_program(Qp: int, Kp: int, Qv: int):
    nc = bacc.Bacc(
        trn_type="TRN2",
        target_bir_lowering=False,
        debug=False,
        num_devices=N_CORES,
    )
    nk = Kp // P
    qt2_ap = nc.dram_tensor("qt2", [P, Qp], BF16, kind="ExternalInput").ap()
    ktd_ap = nc.dram_tensor("ktd", [P, Kp], BF16, kind="ExternalInput").ap()
    kv_ap = nc.dram_tensor("kv", [P, nk * KW], BF16, kind="ExternalInput").ap()
    out_ap = nc.dram_tensor("outT", [KW, Qv], BF16, kind="ExternalOutput").ap()
    with tile.TileContext(nc) as tc, ExitStack() as ctx:
        _emit3(ctx, tc, out_ap, qt2_ap, ktd_ap, kv_ap, Qp, Kp, Qv)
    nc.compile()
    return nc


def shard_inputs(query, key_value, query_coors, key_value_coors):
    import ml_dtypes
    query = np.ascontiguousarray(np.asarray(query), dtype=np.float32)
    key_value = np.ascontiguousarray(np.asarray(key_value), dtype=np.float32)
    qc = np.asarray(query_coors).astype(np.int64)
    kc = np.asarray(key_value_coors).astype(np.int64)
    B = N_CORES
    ids = np.arange(B)
    qs = np.searchsorted(qc, ids, side="left")
    qe = np.searchsorted(qc, ids, side="right")
    ks = np.searchsorted(kc, ids, side="left")
    ke = np.searchsorted(kc, ids, side="right")
    qcnt, kcnt = qe - qs, ke - ks
    Qp = max(_round_up(int(qcnt.max()), P), P)
    Kp = max(_round_up(int(kcnt.max()), P), P)
    Qv = min(_round_up(int(qcnt.max()), 4), Qp)
    nk = Kp // P
    n_hi = nk // 2
    in_maps = []
    for b in range(B):
        qsh = np.zeros((Qp, C), np.float32)
        qsh[: qcnt[b]] = query[qs[b]: qe[b]]
        kvsh = np.zeros((Kp, KW), np.float32)
        kvsh[: kcnt[b], :C] = key_value[ks[b]: ke[b]]
        kvsh[: kcnt[b], C] = 1.0
        kt = kvsh[:, :C].T  # [C, Kp]
        # ktd: rows 0-63 = K^T of every k-tile (T0 mains + tails), rows
        # 64-127 = odd k-tiles' K^T packed at cols [j*P] (T8 mains).
        ktd = np.zeros((P, Kp), np.float32)
        ktd[0:C, :] = kt
        for t in range(n_hi):
            ktd[C:P, t * P:(t + 1) * P] = kt[:, (2 * t + 1) * P:(2 * t + 2) * P]
        # qt2: Q^T duplicated to both row-tile halves.
        qt2 = np.concatenate([qsh.T, qsh.T], axis=0)
        kv_il = kvsh.reshape(nk, P, KW).transpose(1, 0, 2).reshape(P, nk * KW)
        in_maps.append({
            "qt2": np.ascontiguousarray(qt2.astype(ml_dtypes.bfloat16)),
            "ktd": np.ascontiguousarray(ktd.astype(ml_dtypes.bfloat16)),
            "kv": np.ascontiguousarray(kv_il.astype(ml_dtypes.bfloat16)),
        })
    return in_maps, (qs, qe, qcnt), Qp, Kp, Qv


def kernel(query, key_value, query_coors, key_value_coors):
    in_maps, (qs, qe, qcnt), Qp, Kp, Qv = shard_inputs(
        query, key_value, query_coors, key_value_coors
    )
    nc = build_program(Qp, Kp, Qv)
    trace = bool(os.environ.get("XATTN_TRACE"))
    res = run_bass_kernel_spmd(
        nc, in_maps, list(range(N_CORES)), trace=trace,
        trace_cores=list(range(N_CORES)) if trace else None,
    )
    _LAST_RUN["exec_time_ns"] = res.exec_time_ns
    _LAST_RUN["mean_exec_time_ns"] = res.mean_exec_time_ns
    _LAST_RUN["trace"] = res.instructions_and_trace
    _LAST_RUN["results"] = res
    N1 = np.asarray(query).shape[0]
    out = np.zeros((N1, C), np.float32)
    for b in range(N_CORES):
        ot = np.asarray(res.results[b]["outT"], dtype=np.float32)  # [65, Qv]
        n = int(qcnt[b])
        num = ot[:C, :n]
        den = ot[C, :n]
        out[qs[b]: qe[b]] = (num / den[None, :]).T
    return out


# revision 16
# speedup vs baseline: 1.2057x; 1.1358x over previous
"""Per-batch (block-diagonal) cross-attention kernel for Trainium2.

Each query row attends only to key/value rows with the same batch id
(ids in [0, 8), both coor arrays sorted). Batch b -> core b: every core
runs one dense attention block of ~1k queries x ~1k keys, C=64, no
collectives.

v3 design (per core; P=128, Qv = valid queries, Kp = padded keys,
nk = Kp/128, M = main cols, TW = Qv - M tail cols):

  The S matmul has contraction C=64 -- half the PE array. v3 runs it
  in 64x128 row-tiled mode: tile T0 (SBUF partitions 0-63) computes
  S^T for even k-tiles, tile T8 (partitions 64-127) for odd k-tiles,
  concurrently. Q^T is staged twice in SBUF (rows 0-63 and 64-127) so
  each row-tile streams its own copy; even k-tiles' K^T lives at
  partitions 0-63, odd k-tiles' at 64-127 (plus a low copy of every
  tile for the tail matmuls, which all run on T0 so their shared PSUM
  bank is never hit by two tiles at once).

  Per pass j (k-tiles A=2j, B=2j+1):
    - S^T(A) -> ps_A [128, M] (2 banks), S^T(B) -> ps_B (2 banks),
      both in 512-col chunks, T0/T8 interleaved so the array runs 2x.
    - tails: T0-only matmuls into ps_t [128, 2*TW] (1 bank).
    - exp: ACT exact exp for A chunks, DVE Schraudolph for B chunks
      (i16 = S*A + B bitcast to bf16, ~1.5% per-element, cancels in
      softmax normalization); one DVE op for both tails.
    - PV: 128-mode matmuls, kv tile stationary (65 cols: V + ones
      column for the softmax denominator), streaming P^T chunks + the
      tail chunk, accumulated over k-tiles in out^T PSUM chunks
      (c0/c1: [65,512] 1 bank each, c2: [65,TW] 1 bank).
  PSUM: 2+2+1+3 = 8 banks, S single-buffered (exp of pass j frees its
  banks before pass j+1's S needs them).

  Inputs go out on five queues in first-needed order (sync/tensor:
  K^T low/high, scalar/vector: Q^T low/high, gpsimd SWDGE: kv) so S
  of pass 0 starts as soon as the head chunks land and later passes
  never wait. A short warmup matmul train keeps the PE out of its low
  p-state until data arrives without delaying the first real matmul.

  Output: out^T [65, Qv] copied PSUM->SBUF as bf16 (c0 on ACT early,
  c1+c2 on DVE) and DMA'd on two rings; the host does the
  numerator/denominator divide in fp32 and the transpose back.
"""

import os
from contextlib import ExitStack

import numpy as np

import concourse.bacc as bacc
import concourse.mybir as mybir
import concourse.tile as tile
from concourse.bass_utils import run_bass_kernel_spmd

N_CORES = 8
C = 64
P = 128
KW = C + 1  # kv tile width (65: values + ones column)
SCALE = 1.0 / 8.0  # 1/sqrt(C)
F32 = mybir.dt.float32
BF16 = mybir.dt.bfloat16
I16 = mybir.dt.int16

# Schraudolph exp approximation in bf16: exp(s/8) ~= bitcast_bf16(
# int16(s * A + B)). A folds the 1/sqrt(C) score scale into 2^7/ln2.
SCH_A = 184.66496736312366 / 8.0
SCH_B = 16256.0 - 7.42

BANK_F32 = 512  # fp32 elements per PSUM bank (2KB)

_LAST_RUN = {}


def _round_up(x: int, m: int) -> int:
    return -(-x // m) * m


def _emit3(ctx: ExitStack, tc: "tile.TileContext", out_ap, qt2_ap, ktd_ap,
           kv_ap, Qp: int, Kp: int, Qv: int):
    nc = tc.nc
    nk = Kp // P
    n_hi = nk // 2  # number of odd k-tiles (stationaries for T8)
    npass = (nk + 1) // 2

    M = min(Qv, 2 * BANK_F32)
    TW = Qv - M
    m_chunks = [(0, min(M, BANK_F32))]
    if M > BANK_F32:
        m_chunks.append((BANK_F32, M - BANK_F32))
    ncn = len(m_chunks)
    c0w = m_chunks[0][1]

    big = ctx.enter_context(tc.tile_pool(name="big", bufs=1))
    psum_s = ctx.enter_context(tc.tile_pool(name="pss", bufs=1, space="PSUM"))
    psum_o = ctx.enter_context(tc.tile_pool(name="pso", bufs=1, space="PSUM"))
    if TW > 0:
        psum_t = ctx.enter_context(
            tc.tile_pool(name="pst", bufs=1, space="PSUM"))

    # One SBUF tile per DMA op: a tile with several DMA writers makes
    # every reader wait for the LAST writer, gating the whole pipeline
    # on the biggest transfer.
    km = max(P, n_hi * P)  # full-height region end (hi rows packed)
    kt_head = big.tile([P, P], BF16, tag="kth", name="kt_head")
    kt_restA = (big.tile([P, km - P], BF16, tag="ktra", name="kt_restA")
                if km > P else None)
    kt_restB = (big.tile([C, Kp - km], BF16, tag="ktrb", name="kt_restB")
                if Kp > km else None)
    qt_c0 = big.tile([P, c0w], BF16, tag="qt0", name="qt_c0")
    qt_rest = big.tile([P, Qp - c0w], BF16, tag="qtr", name="qt_rest")
    kv_all = big.tile([P, nk * KW], BF16, tag="kv_all", name="kv_all")
    warm = big.tile([C, 512], BF16, tag="warm", name="warm")

    # warm memset rides the idle DVE queue so the gpsimd queue starts
    # the kv DMA immediately.
    nc.vector.memset(warm[:], 0.0)

    # Input DMAs on the three DMA-capable queues, first-needed-first.
    # Beyond the packed hi region only the low rows are transferred.
    nc.sync.dma_start(kt_head[:], ktd_ap[:, 0:P])
    nc.scalar.dma_start(qt_c0[:], qt2_ap[:, 0:c0w])
    if c0w < Qp:
        nc.sync.dma_start(qt_rest[:], qt2_ap[:, c0w:Qp])
    if kt_restA is not None:
        nc.sync.dma_start(kt_restA[:], ktd_ap[:, P:km])
    if kt_restB is not None:
        nc.scalar.dma_start(kt_restB[:], ktd_ap[0:C, km:Kp])
    nc.gpsimd.dma_start(kv_all[:], kv_ap[:, :])

    def kt_lo(t):  # low-rows K^T of k-tile t (T0 mains + all tails)
        if t == 0:
            return kt_head[0:C, 0:P]
        if t * P < km:
            return kt_restA[0:C, (t - 1) * P:t * P]
        return kt_restB[:, t * P - km:(t + 1) * P - km]

    def kta_s(j):  # T0 stationary: even k-tile 2j
        return kt_lo(2 * j)

    def ktb_hi_s(j):  # T8 stationary: odd k-tile 2j+1, high rows
        return (kt_head[C:P, 0:P] if j == 0
                else kt_restA[C:P, (j - 1) * P:j * P])

    def ktb_lo_s(j):  # low copy of odd k-tile 2j+1 (T0 tail matmul)
        return kt_lo(2 * j + 1)

    def qt_sl(rows, off, w):  # query cols [off:off+w], T0 or T8 rows
        r = slice(0, C) if rows == 0 else slice(C, P)
        if off + w <= c0w:
            return qt_c0[r, off:off + w]
        return qt_rest[r, off - c0w:off - c0w + w]

    # Per-chunk P^T tiles: tile-granular dependency tracking means a
    # single [P, M] tile would stall each consumer on every producer.
    pt_c = [[big.tile([P, w], BF16, tag=f"pt{k}c{i}", name=f"pt{k}c{i}")
             for i, (off, w) in enumerate(m_chunks)] for k in range(nk)]
    if TW > 0:
        pt_tails = big.tile([P, nk * TW], BF16, tag="ptt", name="ptt")

    pso_c = [psum_o.tile([KW, w], F32, tag=f"pso{i}", name=f"pso{i}")
             for i, (off, w) in enumerate(m_chunks)]
    if TW > 0:
        pso_tl = psum_o.tile([KW, TW], F32, tag="psotl", name="psotl")
    pso_started = {}

    # PE continuity: the 2.4GHz p-state needs ~3us of gap-free PE
    # execution and resets on ~300ns idle. Warmup matmuls bridge the
    # input-DMA head (into pso_c[0], safe: the first real PV write
    # restarts the accumulation group); standalone LDWEIGHTS groups
    # bridge the exp-latency bubble between phases without needing a
    # PSUM target (each real matmul reloads its own weights anyway).
    def emit_fill(n, src_tile=None):
        for _ in range(n):
            nc.tensor.ldweights(
                warm[:, 0:P] if src_tile is None else src_tile,
                tile_position=(0, 0),
            )

    n_warm = int(os.environ.get("XATTN_WARMUP", "30"))
    for _ in range(n_warm):
        nc.tensor.matmul(
            pso_c[0][0:C, 0:128], lhsT=warm[:, 0:C], rhs=warm[:, 0:128],
            start=True, stop=True, skip_group_check=True,
            tile_position=(0, 0),
        )

    def emit_s_mains(j):
        """S^T mains of pass j: per-chunk PSUM tiles, T0 (even k-tile)
        and T8 (odd) interleaved per chunk so both row-tiles stream
        concurrently."""
        B = 2 * j + 1 if 2 * j + 1 < nk else None
        ps_A = [psum_s.tile([P, w], F32, tag=f"psa{i}", name=f"psA{j}c{i}")
                for i, (off, w) in enumerate(m_chunks)]
        ps_B = ([psum_s.tile([P, w], F32, tag=f"psb{i}", name=f"psB{j}c{i}")
                 for i, (off, w) in enumerate(m_chunks)]
                if B is not None else None)
        for i, (off, w) in enumerate(m_chunks):
            nc.tensor.matmul(
                ps_A[i][:], lhsT=kta_s(j), rhs=qt_sl(0, off, w),
                start=True, stop=True, tile_position=(0, 0),
            )
            if B is not None:
                nc.tensor.matmul(
                    ps_B[i][:], lhsT=ktb_hi_s(j), rhs=qt_sl(1, off, w),
                    start=True, stop=True, tile_position=(C, 0),
                )
        return ps_A, ps_B

    def emit_s_tails(j):
        """S^T tails of pass j, all on T0 (the odd k-tile via its low
        copy) so the shared tails bank never sees two row-tiles at
        once."""
        if TW <= 0:
            return None
        B = 2 * j + 1 if 2 * j + 1 < nk else None
        nt = 2 if B is not None else 1
        ps_t = psum_t.tile([P, nt * TW], F32, tag="pst", name=f"psT{j}")
        nc.tensor.matmul(
            ps_t[:, 0:TW], lhsT=kta_s(j), rhs=qt_sl(0, M, TW),
            start=True, stop=True, tile_position=(0, 0),
        )
        if B is not None:
            nc.tensor.matmul(
                ps_t[:, TW:2 * TW], lhsT=ktb_lo_s(j), rhs=qt_sl(0, M, TW),
                start=True, stop=True, tile_position=(0, 0),
            )
        return ps_t

    def emit_exp(j, ps_A, ps_B, ps_t):
        """exp: ACT exact for A chunks (+ the pass's tails), DVE
        Schraudolph for B chunks; an A-only last pass splits A across
        both engines at the chunk boundary."""
        A = 2 * j
        B = 2 * j + 1 if 2 * j + 1 < nk else None
        for i in range(ncn):
            if i == 1 and B is None:
                nc.vector.tensor_scalar(
                    pt_c[A][i][:].bitcast(I16), ps_A[i][:],
                    SCH_A, SCH_B, mybir.AluOpType.mult, mybir.AluOpType.add,
                )
                continue
            nc.scalar.activation(
                pt_c[A][i][:], ps_A[i][:],
                mybir.ActivationFunctionType.Exp, scale=SCALE,
            )
            if B is not None:
                nc.vector.tensor_scalar(
                    pt_c[B][i][:].bitcast(I16), ps_B[i][:],
                    SCH_A, SCH_B, mybir.AluOpType.mult, mybir.AluOpType.add,
                )
        if TW > 0:
            nt = 2 if B is not None else 1
            nc.scalar.activation(
                pt_tails[:, A * TW:(A + nt) * TW], ps_t[:],
                mybir.ActivationFunctionType.Exp, scale=SCALE,
            )

    def pv_mm(kti, out_tile, key, rhs, stop):
        first = key not in pso_started
        pso_started[key] = True
        nc.tensor.matmul(
            out_tile[:], lhsT=kv_all[:, kti * KW:(kti + 1) * KW], rhs=rhs,
            start=first, stop=stop, skip_group_check=True,
        )

    def emit_pv(j):
        """PV of pair j (128-mode), k-tile-grouped so each kv stationary
        loads once. All exp inputs completed a pass ago. The final
        k-tile is skipped here and fused with the output stage."""
        A = 2 * j
        B = 2 * j + 1 if 2 * j + 1 < nk else None
        for k in (A, B):
            if k is None or k == nk - 1:
                continue
            for i in range(ncn):
                pv_mm(k, pso_c[i], i, pt_c[k][i][:], False)
            if TW > 0:
                pv_mm(k, pso_tl, "tl", pt_tails[:, k * TW:(k + 1) * TW],
                      False)

    # Skewed pipeline with the PE queue order pinned by tile_wait_until
    # (a scheduler-sim-only earliest-start hint, no hardware delay):
    # S(0) | fill | S(1)+tails(0) | PV(0)+fill | S(2)+tails(1) | ...
    # exp(j) runs on ACT/DVE during PV(j-1) so PV never waits; fill
    # groups bridge the exp round-trip so the PE never idles.
    # Rigid total order: every PE group carries a coarse monotone wait
    # (scheduler-sim hint). The sim then serializes groups completely;
    # each group's semaphores batch to "everything scheduled before it",
    # which makes the steady state a rigid, gap-resistant pipeline at
    # the cost of gating pass 0 on the whole input window -- which the
    # ramp-length warmup hides anyway.
    n_fill = int(os.environ.get("XATTN_FILL_N", "8"))
    ms = 1
    with tc.tile_wait_until(ms):
        psA0, psB0 = emit_s_mains(0)
    ps_prev = None
    for j in range(1, npass):
        ms += 1
        with tc.tile_wait_until(ms):
            ps_t_prev = emit_s_tails(j - 1)
            if j == 1:
                emit_exp(0, psA0, psB0, ps_t_prev)
            else:
                emit_exp(j - 1, ps_prev[0], ps_prev[1], ps_t_prev)
            psj = emit_s_mains(j)
        ms += 1
        with tc.tile_wait_until(ms):
            emit_pv(j - 1)
            emit_fill(n_fill)
        ps_prev = psj
    ms += 1
    with tc.tile_wait_until(ms):
        ps_t_last = emit_s_tails(npass - 1)
        if npass == 1:
            emit_exp(0, psA0, psB0, ps_t_last)
        else:
            emit_exp(npass - 1, ps_prev[0], ps_prev[1], ps_t_last)
        emit_pv(npass - 1)
    ms = ms * 1e0

    # Final k-tile PV fused with the output stage: c0 closes first so
    # its cast + DMA overlap the c1/tail matmuls. DMA cannot read PSUM,
    # so stage through SBUF as bf16; host divides num/den in fp32.
    lk = nk - 1
    ow0 = m_chunks[0][1]
    obuf0 = big.tile([KW, ow0], BF16, tag="obuf0", name="obuf0")
    with tc.tile_wait_until(ms):
        pv_mm(lk, pso_c[0], 0, pt_c[lk][0][:], True)
    nc.scalar.activation(
        obuf0[:], pso_c[0][:], mybir.ActivationFunctionType.Copy,
    )
    nc.sync.dma_start(out_ap[:, 0:ow0], obuf0[:])
    if Qv > ow0:
        obuf1 = big.tile([KW, Qv - ow0], BF16, tag="obuf1", name="obuf1")
        if ncn > 1:
            w1 = m_chunks[1][1]
            with tc.tile_wait_until(ms):
                pv_mm(lk, pso_c[1], 1, pt_c[lk][1][:], True)
            nc.vector.tensor_copy(obuf1[:, 0:w1], pso_c[1][:])
        if TW > 0:
            with tc.tile_wait_until(ms):
                pv_mm(lk, pso_tl, "tl", pt_tails[:, lk * TW:(lk + 1) * TW],
                      True)
            nc.vector.tensor_copy(obuf1[:, M - ow0:Qv - ow0], pso_tl[:])
        nc.gpsimd.dma_start(out_ap[:, ow0:Qv], obuf1[:])


def build_program(Qp: int, Kp: int, Qv: int):
    nc = bacc.Bacc(
        trn_type="TRN2",
        target_bir_lowering=False,
        debug=False,
        num_devices=N_CORES,
    )
    nk = Kp // P
    qt2_ap = nc.dram_tensor("qt2", [P, Qp], BF16, kind="ExternalInput").ap()
    ktd_ap = nc.dram_tensor("ktd", [P, Kp], BF16, kind="ExternalInput").ap()
    kv_ap = nc.dram_tensor("kv", [P, nk * KW], BF16, kind="ExternalInput").ap()
    out_ap = nc.dram_tensor("outT", [KW, Qv], BF16, kind="ExternalOutput").ap()
    with tile.TileContext(nc) as tc, ExitStack() as ctx:
        _emit3(ctx, tc, out_ap, qt2_ap, ktd_ap, kv_ap, Qp, Kp, Qv)
    nc.compile()
    return nc


def shard_inputs(query, key_value, query_coors, key_value_coors):
    import ml_dtypes
    query = np.ascontiguousarray(np.asarray(query), dtype=np.float32)
    key_value = np.ascontiguousarray(np.asarray(key_value), dtype=np.float32)
    qc = np.asarray(query_coors).astype(np.int64)
    kc = np.asarray(key_value_coors).astype(np.int64)
    B = N_CORES
    ids = np.arange(B)
    qs = np.searchsorted(qc, ids, side="left")
    qe = np.searchsorted(qc, ids, side="right")
    ks = np.searchsorted(kc, ids, side="left")
    ke = np.searchsorted(kc, ids, side="right")
    qcnt, kcnt = qe - qs, ke - ks
    Qp = max(_round_up(int(qcnt.max()), P), P)
    Kp = max(_round_up(int(kcnt.max()), P), P)
    Qv = min(_round_up(int(qcnt.max()), 4), Qp)
    nk = Kp // P
    n_hi = nk // 2
    in_maps = []
    for b in range(B):
        qsh = np.zeros((Qp, C), np.float32)
        qsh[: qcnt[b]] = query[qs[b]: qe[b]]
        kvsh = np.zeros((Kp, KW), np.float32)
        kvsh[: kcnt[b], :C] = key_value[ks[b]: ke[b]]
        kvsh[: kcnt[b], C] = 1.0
        kt = kvsh[:, :C].T  # [C, Kp]
        # ktd: rows 0-63 = K^T of every k-tile (T0 mains + tails), rows
        # 64-127 = odd k-tiles' K^T packed at cols [j*P] (T8 mains).
        ktd = np.zeros((P, Kp), np.float32)
        ktd[0:C, :] = kt
        for t in range(n_hi):
            ktd[C:P, t * P:(t + 1) * P] = kt[:, (2 * t + 1) * P:(2 * t + 2) * P]
        # qt2: Q^T duplicated to both row-tile halves.
        qt2 = np.concatenate([qsh.T, qsh.T], axis=0)
        kv_il = kvsh.reshape(nk, P, KW).transpose(1, 0, 2).reshape(P, nk * KW)
        in_maps.append({
            "qt2": np.ascontiguousarray(qt2.astype(ml_dtypes.bfloat16)),
            "ktd": np.ascontiguousarray(ktd.astype(ml_dtypes.bfloat16)),
            "kv": np.ascontiguousarray(kv_il.astype(ml_dtypes.bfloat16)),
        })
    return in_maps, (qs, qe, qcnt), Qp, Kp, Qv


def kernel(query, key_value, query_coors, key_value_coors):
    in_maps, (qs, qe, qcnt), Qp, Kp, Qv = shard_inputs(
        query, key_value, query_coors, key_value_coors
    )
    nc = build_program(Qp, Kp, Qv)
    trace = bool(os.environ.get("XATTN_TRACE"))
    res = run_bass_kernel_spmd(
        nc, in_maps, list(range(N_CORES)), trace=trace,
        trace_cores=list(range(N_CORES)) if trace else None,
    )
    _LAST_RUN["exec_time_ns"] = res.exec_time_ns
    _LAST_RUN["mean_exec_time_ns"] = res.mean_exec_time_ns
    _LAST_RUN["trace"] = res.instructions_and_trace
    _LAST_RUN["results"] = res
    N1 = np.asarray(query).shape[0]
    out = np.zeros((N1, C), np.float32)
    for b in range(N_CORES):
        ot = np.asarray(res.results[b]["outT"], dtype=np.float32)  # [65, Qv]
        n = int(qcnt[b])
        num = ot[:C, :n]
        den = ot[C, :n]
        out[qs[b]: qe[b]] = (num / den[None, :]).T
    return out
